# revision 20
# baseline (speedup 1.0000x reference)
"""Trainium2 Bass kernel: single-head attention layer (QKV proj + tanh-squashed
scaled-dot-product softmax attention), data-parallel over batch on 8 NeuronCores.

Reference math (per batch b):
    Q = x Wq + bq ; K = x Wk + bk ; V = x Wv + bv          # [S, E]
    scores  = tanh((Q K^T) / sqrt(E))                      # [S, S]
    weights = softmax(scores, axis=-1)
    context = weights V

Device-side strategy (per core, S=2048, D=E=512):
    - host supplies xT = x[b].T [D, S] plus bias re-layouts (layout prep only);
      inputs are DVE-cast to bf16 once on-chip (PE runs bf16 at full rate).
    - V in natural [S, E] layout, Q^T / K^T in [E, S] layout; biases are added
      on the PSUM->SBUF evacuation (DVE), not on the PE.
    - scoresT tiles [128k, S] = K^T.T @ Q^T -> tanh -> exp (ScalarE; softmax
      needs no max subtraction because tanh bounds scores to [-1, 1]).
    - P^T (unnormalized numerator) kept in SBUF as bf16; a running DVE sum of
      P^T tiles + one gpsimd partition_all_reduce gives the softmax
      denominators with zero PE cost; reciprocal_approx_fast + gpsimd
      partition_broadcast make the recip row available on all partitions.
    - both outputs are produced transposed so normalization is a free-axis
      tensor_tensor multiply: weightsT = P^T * rbc (bf16 -> DMA out; host
      transposes + upcasts, both exact), ctx^T = (V.T-chunk @ P^T) * rbc
      (f32 -> DMA out; host transposes).
    - one PSUM pool (4 x [128,1024] rotating buffers, 8 banks) serves QKV,
      scores and ctx^T accumulations — no pool-transition drains on the PE
      critical path; input DMAs are chunked/ordered so the first V matmul
      group unblocks after ~0.5MB.
"""

import sys

sys.path.insert(0, "/opt/trn_rl_repo")

import numpy as np

import concourse.bacc as bacc
import concourse.bass as bass
import concourse.tile as tile
from concourse import bass_isa, mybir
from concourse.bass_utils import run_bass_kernel_spmd

F32 = mybir.dt.float32
BF16 = mybir.dt.bfloat16
AF = mybir.ActivationFunctionType

B = 8          # batch (one per core)
S = 2048       # sequence length
D = 512        # d_in
E = 512        # attention size
P = 128        # SBUF partitions
NS = S // P    # 16 seq tiles of 128
ND = D // P    # 4 contraction chunks
NE = E // P    # 4 e tiles
NQ = S // 512  # 4 moving chunks of 512 along seq
SCALE = 1.0 / float(np.sqrt(E))

N_CORES = 8


def _build():
    nc = bacc.Bacc("TRN2", target_bir_lowering=False, debug=False,
                   num_devices=N_CORES)

    xt_e = nc.declare_dram_parameter("xt", [D, S], F32, isOutput=False)
    wq_e = nc.declare_dram_parameter("wq", [D, E], F32, isOutput=False)
    wk_e = nc.declare_dram_parameter("wk", [D, E], F32, isOutput=False)
    wv_e = nc.declare_dram_parameter("wv", [D, E], F32, isOutput=False)
    # bias re-layouts (host-prepped): [128, 4] with column i = b[128*i:128*(i+1)]
    bqc_e = nc.declare_dram_parameter("bqc", [P, NE], F32, isOutput=False)
    bkc_e = nc.declare_dram_parameter("bkc", [P, NE], F32, isOutput=False)
    bv_e = nc.declare_dram_parameter("bv", [1, E], F32, isOutput=False)
    wt_e = nc.declare_dram_parameter("wt_out", [S, S], BF16, isOutput=True)
    cx_e = nc.declare_dram_parameter("ctxT_out", [E, S], F32, isOutput=True)


    with tile.TileContext(nc) as tc:
        with tc.tile_pool(name="persist", bufs=1) as persist, \
             tc.tile_pool(name="psA", bufs=1, space="PSUM") as psA:
            qT = [persist.tile([P, S], BF16, tag=f"qT{i}", name=f"qT{i}")
                  for i in range(NE)]
            kT = [persist.tile([P, S], BF16, tag=f"kT{i}", name=f"kT{i}")
                  for i in range(NE)]
            v = [persist.tile([P, E], BF16, tag=f"v{i}", name=f"v{i}")
                 for i in range(NS)]
            pT = [persist.tile([P, S], BF16, tag=f"pT{i}", name=f"pT{i}")
                  for i in range(NS)]

            # ---------------- phase 1: QKV projections ----------------
            # DMA f32 inputs into small staging tiles, DVE-cast to bf16
            # operand tiles (PE runs bf16 at full rate; rounding noise is
            # dominated by the bf16 P^T/weights path anyway).
            with tc.tile_pool(name="ph1", bufs=1) as ph1:
                w_sb = {}
                for nm in ("v", "q", "k"):
                    w_sb[nm] = [ph1.tile([P, E], BF16, tag=f"w{nm}{d}",
                                         name=f"w{nm}{d}") for d in range(ND)]
                xt = [ph1.tile([P, S], BF16, tag=f"xt{i}", name=f"xt{i}")
                      for i in range(ND)]

                def load_cast(dst, src_ap):
                    st = ph1.tile([P, 512], F32, tag="stage", bufs=4)
                    nc.sync.dma_start(out=st, in_=src_ap)
                    nc.vector.tensor_copy(dst, st)

                # wv[d] + xt[d] col-chunk 0 first so V matmuls start early
                for d in range(ND):
                    load_cast(w_sb["v"][d], wv_e.ap()[d * P:(d + 1) * P, :])
                    load_cast(xt[d][:, 0:512],
                              xt_e.ap()[d * P:(d + 1) * P, 0:512])
                bqc = ph1.tile([P, NE], F32, tag="bqc")
                nc.sync.dma_start(out=bqc, in_=bqc_e.ap())
                bkc = ph1.tile([P, NE], F32, tag="bkc")
                nc.sync.dma_start(out=bkc, in_=bkc_e.ap())
                # bv broadcast across all 128 partitions
                bvb = ph1.tile([P, E], F32, tag="bvb")
                src = bv_e.ap()
                nc.sync.dma_start(
                    out=bvb,
                    in_=bass.AP(tensor=src.tensor, offset=src.offset,
                                ap=[[0, P], [1, E]]))
                for cc in range(1, NQ):
                    for dc in range(ND):
                        load_cast(xt[dc][:, cc * 512:(cc + 1) * 512],
                                  xt_e.ap()[dc * P:(dc + 1) * P,
                                            cc * 512:(cc + 1) * 512])
                for nm, we in (("q", wq_e), ("k", wk_e)):
                    for d in range(ND):
                        load_cast(w_sb[nm][d], we.ap()[d * P:(d + 1) * P, :])

                # V first (scores depend only on Q^T/K^T below)
                for si in range(NS):
                    ps = psA.tile([P, 1024], F32, tag="ps_mm", bufs=4)
                    for dc in range(ND):
                        nc.tensor.matmul(
                            ps[:, :E],
                            lhsT=xt[dc][:, si * P:(si + 1) * P],
                            rhs=w_sb["v"][dc],
                            start=(dc == 0), stop=(dc == ND - 1))
                    nc.vector.tensor_add(v[si], ps[:, :E], bvb)

                # Q^T, K^T in [E, S] layout; bias added on evacuation
                for nm, dst, bc in (("q", qT, bqc), ("k", kT, bkc)):
                    for ei in range(NE):
                        for sj in range(NQ):
                            ps = psA.tile([P, 1024], F32, tag="ps_mm", bufs=4)
                            for dc in range(ND):
                                nc.tensor.matmul(
                                    ps[:, :512],
                                    lhsT=w_sb[nm][dc][:, ei * P:(ei + 1) * P],
                                    rhs=xt[dc][:, sj * 512:(sj + 1) * 512],
                                    start=(dc == 0), stop=(dc == ND - 1))
                            nc.vector.tensor_scalar_add(
                                out=dst[ei][:, sj * 512:(sj + 1) * 512],
                                in0=ps[:, :512],
                                scalar1=bc[:, ei:ei + 1])

            # ---------------- phase 2: scoresT -> P^T ----------------
            with tc.tile_pool(name="ph23", bufs=1) as ph23:
                # running sum of pT tiles (DVE, otherwise idle in phase 2);
                # one gpsimd partition_all_reduce at the end gives row sums
                acc = ph23.tile([P, S], F32, tag="acc")

                def ctxT_mms(halves, ei, kt):
                    for h in range(2):
                        for qj in (2 * h, 2 * h + 1):
                            col = (qj - 2 * h) * 512
                            nc.tensor.matmul(
                                halves[h][:, col:col + 512],
                                lhsT=v[kt][:, ei * P:(ei + 1) * P],
                                rhs=pT[kt][:, qj * 512:(qj + 1) * 512],
                                start=(kt == 0), stop=(kt == NS - 1))

                for kt in range(NS):
                    tt = ph23.tile([P, S], BF16, tag="t_tanh", bufs=2)
                    for h in range(2):
                        ps = psA.tile([P, 1024], F32, tag="ps_mm", bufs=4)
                        for qj in (2 * h, 2 * h + 1):
                            col = (qj - 2 * h) * 512
                            for ec in range(NE):
                                nc.tensor.matmul(
                                    ps[:, col:col + 512],
                                    lhsT=kT[ec][:, kt * P:(kt + 1) * P],
                                    rhs=qT[ec][:, qj * 512:(qj + 1) * 512],
                                    start=(ec == 0), stop=(ec == NE - 1))
                        nc.scalar.activation(
                            tt[:, h * 1024:(h + 1) * 1024], ps,
                            func=AF.Tanh, scale=SCALE)
                    nc.scalar.activation(pT[kt], tt, func=AF.Exp)
                    if kt == 0:
                        nc.vector.tensor_copy(acc, pT[0])
                    else:
                        nc.vector.tensor_add(acc, acc, pT[kt])

                # ------------ softmax denominators (off the PE) ------------
                rs_all = ph23.tile([P, S], F32, tag="rs_all")
                nc.gpsimd.partition_all_reduce(
                    rs_all, acc, channels=P, reduce_op=bass_isa.ReduceOp.add)
                rec_row = ph23.tile([1, S], F32, tag="rec_row")
                nc.vector.reciprocal_approx_fast(rec_row, rs_all[0:1, :])
                rec_bf = ph23.tile([1, S], BF16, tag="rec_bf")
                nc.vector.tensor_copy(rec_bf, rec_row)
                rbc = ph23.tile([P, S], BF16, tag="rbc")
                nc.gpsimd.partition_broadcast(rbc, rec_bf)
                rbc32 = ph23.tile([P, S], F32, tag="rbc32")
                nc.gpsimd.partition_broadcast(rbc32, rec_row)

                # ------------ phase 3: context^T + weights out ------------
                # ctx^T[e, q] = sum_k V[k, e] P^T[k, q]; both outputs
                # normalize along the FREE axis against the rbc broadcast.
                def evac_ctxT(halves, ei):
                    cx_sb = ph23.tile([P, S], F32, tag="ctx_stage", bufs=2,
                                      name=f"cx_sb{ei}")
                    for h in range(2):
                        nc.vector.tensor_mul(
                            cx_sb[:, h * 1024:(h + 1) * 1024], halves[h],
                            rbc32[:, h * 1024:(h + 1) * 1024])
                    nc.sync.dma_start(
                        out=cx_e.ap()[ei * P:(ei + 1) * P, :], in_=cx_sb)

                WT_SCHED = {0: range(0, 6), 1: range(6, 12), 2: range(12, 15),
                            3: range(15, 16)}
                for ei in range(NE):
                    halves = [psA.tile([P, 1024], F32, tag="ps_mm", bufs=4,
                                       name=f"psctxT{ei}_{h}") for h in range(2)]
                    for kt in range(NS):
                        ctxT_mms(halves, ei, kt)
                    evac_ctxT(halves, ei)
                    for kt in WT_SCHED[ei]:
                        wt_sb = ph23.tile([P, S], BF16, tag="wt_stage", bufs=3)
                        nc.vector.tensor_mul(wt_sb, pT[kt], rbc)
                        nc.sync.dma_start(
                            out=wt_e.ap()[kt * P:(kt + 1) * P, :], in_=wt_sb)

    nc.finalize()
    return nc


_NC_CACHE = None
_LAST_RESULT = None


def _get_nc():
    global _NC_CACHE
    if _NC_CACHE is None:
        _NC_CACHE = _build()
    return _NC_CACHE


def kernel(x, Wq, bq, Wk, bk, Wv, bv):
    x = np.asarray(x, dtype=np.float32)
    Wq = np.asarray(Wq, dtype=np.float32)
    Wk = np.asarray(Wk, dtype=np.float32)
    Wv = np.asarray(Wv, dtype=np.float32)
    bq = np.asarray(bq, dtype=np.float32).reshape(E)
    bk = np.asarray(bk, dtype=np.float32).reshape(E)
    bv = np.asarray(bv, dtype=np.float32).reshape(1, E)

    nc = _get_nc()
    bqc = np.ascontiguousarray(bq.reshape(NE, P).T)  # [128, 4]
    bkc = np.ascontiguousarray(bk.reshape(NE, P).T)
    shared = {"wq": Wq, "wk": Wk, "wv": Wv, "bqc": bqc, "bkc": bkc, "bv": bv}
    in_maps = [
        {"xt": np.ascontiguousarray(x[b].T), **shared} for b in range(N_CORES)
    ]
    res = run_bass_kernel_spmd(nc, in_maps, list(range(N_CORES)))
    global _LAST_RESULT
    _LAST_RESULT = res

    context = np.stack([
        np.ascontiguousarray(res.results[b]["ctxT_out"].T) for b in range(N_CORES)
    ])
    # weightsT is [k, q] bf16; transpose back and upcast (both exact)
    weights = np.stack([
        np.ascontiguousarray(res.results[b]["wt_out"].T).astype(np.float32)
        for b in range(N_CORES)
    ])
    return context, weights


# revision 21
# speedup vs baseline: 1.0044x; 1.0044x over previous
"""Trainium2 Bass kernel: single-head attention layer (QKV proj + tanh-squashed
scaled-dot-product softmax attention), data-parallel over batch on 8 NeuronCores.

Reference math (per batch b):
    Q = x Wq + bq ; K = x Wk + bk ; V = x Wv + bv          # [S, E]
    scores  = tanh((Q K^T) / sqrt(E))                      # [S, S]
    weights = softmax(scores, axis=-1)
    context = weights V

Device-side strategy (per core, S=2048, D=E=512):
    - host supplies xT = x[b].T [D, S] plus bias re-layouts (layout prep only);
      inputs are DVE-cast to bf16 once on-chip (PE runs bf16 at full rate).
    - V in natural [S, E] layout, Q^T / K^T in [E, S] layout; biases are added
      on the PSUM->SBUF evacuation (DVE), not on the PE.
    - scoresT tiles [128k, S] = K^T.T @ Q^T -> tanh -> exp (ScalarE; softmax
      needs no max subtraction because tanh bounds scores to [-1, 1]).
    - P^T (unnormalized numerator) kept in SBUF as bf16; a running DVE sum of
      P^T tiles + one gpsimd partition_all_reduce gives the softmax
      denominators with zero PE cost; reciprocal_approx_fast + gpsimd
      partition_broadcast make the recip row available on all partitions.
    - both outputs are produced transposed so normalization is a free-axis
      tensor_tensor multiply: weightsT = P^T * rbc (bf16 -> DMA out; host
      transposes + upcasts, both exact), ctx^T = (V.T-chunk @ P^T) * rbc
      (f32 -> DMA out; host transposes).
    - one PSUM pool (4 x [128,1024] rotating buffers, 8 banks) serves QKV,
      scores and ctx^T accumulations — no pool-transition drains on the PE
      critical path; input DMAs are chunked/ordered so the first V matmul
      group unblocks after ~0.5MB.
"""

import sys

sys.path.insert(0, "/opt/trn_rl_repo")

import numpy as np

import concourse.bacc as bacc
import concourse.bass as bass
import concourse.tile as tile
from concourse import bass_isa, mybir
from concourse.bass_utils import run_bass_kernel_spmd

F32 = mybir.dt.float32
BF16 = mybir.dt.bfloat16
AF = mybir.ActivationFunctionType

B = 8          # batch (one per core)
S = 2048       # sequence length
D = 512        # d_in
E = 512        # attention size
P = 128        # SBUF partitions
NS = S // P    # 16 seq tiles of 128
ND = D // P    # 4 contraction chunks
NE = E // P    # 4 e tiles
NQ = S // 512  # 4 moving chunks of 512 along seq
SCALE = 1.0 / float(np.sqrt(E))

N_CORES = 8


def _build():
    nc = bacc.Bacc("TRN2", target_bir_lowering=False, debug=False,
                   num_devices=N_CORES)

    xt_e = nc.declare_dram_parameter("xt", [D, S], F32, isOutput=False)
    wq_e = nc.declare_dram_parameter("wq", [D, E], F32, isOutput=False)
    wk_e = nc.declare_dram_parameter("wk", [D, E], F32, isOutput=False)
    wv_e = nc.declare_dram_parameter("wv", [D, E], F32, isOutput=False)
    # bias re-layouts (host-prepped): [128, 4] with column i = b[128*i:128*(i+1)]
    bqc_e = nc.declare_dram_parameter("bqc", [P, NE], F32, isOutput=False)
    bkc_e = nc.declare_dram_parameter("bkc", [P, NE], F32, isOutput=False)
    bv_e = nc.declare_dram_parameter("bv", [1, E], F32, isOutput=False)
    wt_e = nc.declare_dram_parameter("wt_out", [S, S], BF16, isOutput=True)
    cx_e = nc.declare_dram_parameter("ctxT_out", [E, S], F32, isOutput=True)


    with tile.TileContext(nc) as tc:
        with tc.tile_pool(name="persist", bufs=1) as persist, \
             tc.tile_pool(name="psA", bufs=1, space="PSUM") as psA:
            qT = [persist.tile([P, S], BF16, tag=f"qT{i}", name=f"qT{i}")
                  for i in range(NE)]
            kT = [persist.tile([P, S], BF16, tag=f"kT{i}", name=f"kT{i}")
                  for i in range(NE)]
            v = [persist.tile([P, E], BF16, tag=f"v{i}", name=f"v{i}")
                 for i in range(NS)]
            pT = [persist.tile([P, S], BF16, tag=f"pT{i}", name=f"pT{i}")
                  for i in range(NS)]

            # ---------------- phase 1: QKV projections ----------------
            # DMA f32 inputs into small staging tiles, DVE-cast to bf16
            # operand tiles (PE runs bf16 at full rate; rounding noise is
            # dominated by the bf16 P^T/weights path anyway).
            with tc.tile_pool(name="ph1", bufs=1) as ph1:
                w_sb = {}
                for nm in ("v", "q", "k"):
                    w_sb[nm] = [ph1.tile([P, E], BF16, tag=f"w{nm}{d}",
                                         name=f"w{nm}{d}") for d in range(ND)]
                xt = [ph1.tile([P, S], BF16, tag=f"xt{i}", name=f"xt{i}")
                      for i in range(ND)]

                def load_cast(dst, src_ap):
                    st = ph1.tile([P, 512], F32, tag="stage", bufs=4)
                    nc.sync.dma_start(out=st, in_=src_ap)
                    nc.vector.tensor_copy(dst, st)

                # wv[d] + xt[d] col-chunk 0 first so V matmuls start early
                for d in range(ND):
                    load_cast(w_sb["v"][d], wv_e.ap()[d * P:(d + 1) * P, :])
                    load_cast(xt[d][:, 0:512],
                              xt_e.ap()[d * P:(d + 1) * P, 0:512])
                bqc = ph1.tile([P, NE], F32, tag="bqc")
                nc.sync.dma_start(out=bqc, in_=bqc_e.ap())
                bkc = ph1.tile([P, NE], F32, tag="bkc")
                nc.sync.dma_start(out=bkc, in_=bkc_e.ap())
                # bv broadcast across all 128 partitions
                bvb = ph1.tile([P, E], F32, tag="bvb")
                src = bv_e.ap()
                nc.sync.dma_start(
                    out=bvb,
                    in_=bass.AP(tensor=src.tensor, offset=src.offset,
                                ap=[[0, P], [1, E]]))
                for cc in range(1, NQ):
                    for dc in range(ND):
                        load_cast(xt[dc][:, cc * 512:(cc + 1) * 512],
                                  xt_e.ap()[dc * P:(dc + 1) * P,
                                            cc * 512:(cc + 1) * 512])
                for nm, we in (("q", wq_e), ("k", wk_e)):
                    for d in range(ND):
                        load_cast(w_sb[nm][d], we.ap()[d * P:(d + 1) * P, :])

                # V first (scores depend only on Q^T/K^T below)
                for si in range(NS):
                    ps = psA.tile([P, 1024], F32, tag="ps_mm", bufs=4)
                    for dc in range(ND):
                        nc.tensor.matmul(
                            ps[:, :E],
                            lhsT=xt[dc][:, si * P:(si + 1) * P],
                            rhs=w_sb["v"][dc],
                            start=(dc == 0), stop=(dc == ND - 1))
                    nc.vector.tensor_add(v[si], ps[:, :E], bvb)

                # Q^T, K^T in [E, S] layout; bias added on evacuation
                for nm, dst, bc in (("q", qT, bqc), ("k", kT, bkc)):
                    for ei in range(NE):
                        for sj in range(NQ):
                            ps = psA.tile([P, 1024], F32, tag="ps_mm", bufs=4)
                            for dc in range(ND):
                                nc.tensor.matmul(
                                    ps[:, :512],
                                    lhsT=w_sb[nm][dc][:, ei * P:(ei + 1) * P],
                                    rhs=xt[dc][:, sj * 512:(sj + 1) * 512],
                                    start=(dc == 0), stop=(dc == ND - 1))
                            nc.vector.tensor_scalar_add(
                                out=dst[ei][:, sj * 512:(sj + 1) * 512],
                                in0=ps[:, :512],
                                scalar1=bc[:, ei:ei + 1])

            # ---------------- phase 2: scoresT -> P^T ----------------
            with tc.tile_pool(name="ph23", bufs=1) as ph23:
                # running sum of pT tiles (DVE, otherwise idle in phase 2);
                # one gpsimd partition_all_reduce at the end gives row sums
                acc = ph23.tile([P, S], F32, tag="acc")

                def ctxT_mms(halves, ei, kt):
                    for h in range(2):
                        for qj in (2 * h, 2 * h + 1):
                            col = (qj - 2 * h) * 512
                            nc.tensor.matmul(
                                halves[h][:, col:col + 512],
                                lhsT=v[kt][:, ei * P:(ei + 1) * P],
                                rhs=pT[kt][:, qj * 512:(qj + 1) * 512],
                                start=(kt == 0), stop=(kt == NS - 1))

                for kt in range(NS):
                    tt = ph23.tile([P, S], BF16, tag="t_tanh", bufs=2)
                    for h in range(2):
                        ps = psA.tile([P, 1024], F32, tag="ps_mm", bufs=4)
                        for qj in (2 * h, 2 * h + 1):
                            col = (qj - 2 * h) * 512
                            for ec in range(NE):
                                nc.tensor.matmul(
                                    ps[:, col:col + 512],
                                    lhsT=kT[ec][:, kt * P:(kt + 1) * P],
                                    rhs=qT[ec][:, qj * 512:(qj + 1) * 512],
                                    start=(ec == 0), stop=(ec == NE - 1))
                        nc.scalar.activation(
                            tt[:, h * 1024:(h + 1) * 1024], ps,
                            func=AF.Tanh, scale=SCALE)
                    nc.scalar.activation(pT[kt], tt, func=AF.Exp)
                    if kt == 0:
                        nc.vector.tensor_copy(acc, pT[0])
                    else:
                        nc.vector.tensor_add(acc, acc, pT[kt])

                # ------------ softmax denominators (off the PE) ------------
                rs_all = ph23.tile([P, S], F32, tag="rs_all")
                nc.gpsimd.partition_all_reduce(
                    rs_all, acc, channels=P, reduce_op=bass_isa.ReduceOp.add)
                rec_row = ph23.tile([1, S], F32, tag="rec_row")
                nc.vector.reciprocal_approx_fast(rec_row, rs_all[0:1, :])
                rec_bf = ph23.tile([1, S], BF16, tag="rec_bf")
                nc.vector.tensor_copy(rec_bf, rec_row)
                rbc = ph23.tile([P, S], BF16, tag="rbc")
                nc.gpsimd.partition_broadcast(rbc, rec_bf)

                # ------------ phase 3: context^T + weights out ------------
                # ctx^T[e, q] = sum_k V[k, e] P^T[k, q]; both outputs
                # normalize along the FREE axis against the rbc broadcast.
                def evac_ctxT(halves, ei):
                    cx_sb = ph23.tile([P, S], F32, tag="ctx_stage", bufs=2,
                                      name=f"cx_sb{ei}")
                    for h in range(2):
                        nc.vector.tensor_mul(
                            cx_sb[:, h * 1024:(h + 1) * 1024], halves[h],
                            rbc[:, h * 1024:(h + 1) * 1024])
                    nc.sync.dma_start(
                        out=cx_e.ap()[ei * P:(ei + 1) * P, :], in_=cx_sb)

                WT_SCHED = {0: range(0, 6), 1: range(6, 12), 2: range(12, 15),
                            3: range(15, 16)}
                for ei in range(NE):
                    halves = [psA.tile([P, 1024], F32, tag="ps_mm", bufs=4,
                                       name=f"psctxT{ei}_{h}") for h in range(2)]
                    for kt in range(NS):
                        ctxT_mms(halves, ei, kt)
                    evac_ctxT(halves, ei)
                    for kt in WT_SCHED[ei]:
                        wt_sb = ph23.tile([P, S], BF16, tag="wt_stage", bufs=3)
                        nc.vector.tensor_mul(wt_sb, pT[kt], rbc)
                        nc.sync.dma_start(
                            out=wt_e.ap()[kt * P:(kt + 1) * P, :], in_=wt_sb)

    nc.finalize()
    return nc


_NC_CACHE = None
_LAST_RESULT = None


def _get_nc():
    global _NC_CACHE
    if _NC_CACHE is None:
        _NC_CACHE = _build()
    return _NC_CACHE


def kernel(x, Wq, bq, Wk, bk, Wv, bv):
    x = np.asarray(x, dtype=np.float32)
    Wq = np.asarray(Wq, dtype=np.float32)
    Wk = np.asarray(Wk, dtype=np.float32)
    Wv = np.asarray(Wv, dtype=np.float32)
    bq = np.asarray(bq, dtype=np.float32).reshape(E)
    bk = np.asarray(bk, dtype=np.float32).reshape(E)
    bv = np.asarray(bv, dtype=np.float32).reshape(1, E)

    nc = _get_nc()
    bqc = np.ascontiguousarray(bq.reshape(NE, P).T)  # [128, 4]
    bkc = np.ascontiguousarray(bk.reshape(NE, P).T)
    shared = {"wq": Wq, "wk": Wk, "wv": Wv, "bqc": bqc, "bkc": bkc, "bv": bv}
    in_maps = [
        {"xt": np.ascontiguousarray(x[b].T), **shared} for b in range(N_CORES)
    ]
    res = run_bass_kernel_spmd(nc, in_maps, list(range(N_CORES)))
    global _LAST_RESULT
    _LAST_RESULT = res

    context = np.stack([
        np.ascontiguousarray(res.results[b]["ctxT_out"].T) for b in range(N_CORES)
    ])
    # weightsT is [k, q] bf16; transpose back and upcast (both exact)
    weights = np.stack([
        np.ascontiguousarray(res.results[b]["wt_out"].T).astype(np.float32)
        for b in range(N_CORES)
    ])
    return context, weights


# revision 23
# speedup vs baseline: 1.0405x; 1.0360x over previous
"""Trainium2 Bass kernel: single-head attention layer (QKV proj + tanh-squashed
scaled-dot-product softmax attention), data-parallel over batch on 8 NeuronCores.

Reference math (per batch b):
    Q = x Wq + bq ; K = x Wk + bk ; V = x Wv + bv          # [S, E]
    scores  = tanh((Q K^T) / sqrt(E))                      # [S, S]
    weights = softmax(scores, axis=-1)
    context = weights V

Device-side strategy (per core, S=2048, D=E=512):
    - host supplies xT = x[b].T [D, S] plus bias re-layouts (layout prep only);
      inputs are DVE-cast to bf16 once on-chip (PE runs bf16 at full rate).
    - V in natural [S, E] layout, Q^T / K^T in [E, S] layout; biases are added
      on the PSUM->SBUF evacuation (DVE), not on the PE.
    - scoresT tiles [128k, S] = K^T.T @ Q^T -> tanh -> exp (ScalarE; softmax
      needs no max subtraction because tanh bounds scores to [-1, 1]).
    - P^T (unnormalized numerator) kept in SBUF as bf16; a running DVE sum of
      P^T tiles + one gpsimd partition_all_reduce gives the softmax
      denominators with zero PE cost; reciprocal_approx_fast + gpsimd
      partition_broadcast make the recip row available on all partitions.
    - both outputs are produced transposed so normalization is a free-axis
      tensor_tensor multiply: weightsT = P^T * rbc (bf16 -> DMA out; host
      transposes + upcasts, both exact), ctx^T = (V.T-chunk @ P^T) * rbc
      (f32 -> DMA out; host transposes).
    - one PSUM pool (4 x [128,1024] rotating buffers, 8 banks) serves QKV,
      scores and ctx^T accumulations — no pool-transition drains on the PE
      critical path; input DMAs are chunked/ordered so the first V matmul
      group unblocks after ~0.5MB.
"""

import sys

sys.path.insert(0, "/opt/trn_rl_repo")

import numpy as np

import concourse.bacc as bacc
import concourse.bass as bass
import concourse.tile as tile
from concourse import bass_isa, mybir
from concourse.bass_utils import run_bass_kernel_spmd

F32 = mybir.dt.float32
BF16 = mybir.dt.bfloat16
AF = mybir.ActivationFunctionType

B = 8          # batch (one per core)
S = 2048       # sequence length
D = 512        # d_in
E = 512        # attention size
P = 128        # SBUF partitions
NS = S // P    # 16 seq tiles of 128
ND = D // P    # 4 contraction chunks
NE = E // P    # 4 e tiles
NQ = S // 512  # 4 moving chunks of 512 along seq
SCALE = 1.0 / float(np.sqrt(E))

N_CORES = 8


def _build():
    nc = bacc.Bacc("TRN2", target_bir_lowering=False, debug=False,
                   num_devices=N_CORES)

    xt_e = nc.declare_dram_parameter("xt", [D, S], F32, isOutput=False)
    wq_e = nc.declare_dram_parameter("wq", [D, E], F32, isOutput=False)
    wk_e = nc.declare_dram_parameter("wk", [D, E], F32, isOutput=False)
    wv_e = nc.declare_dram_parameter("wv", [D, E], F32, isOutput=False)
    # bias re-layouts (host-prepped): [128, 4] with column i = b[128*i:128*(i+1)]
    bqc_e = nc.declare_dram_parameter("bqc", [P, NE], F32, isOutput=False)
    bkc_e = nc.declare_dram_parameter("bkc", [P, NE], F32, isOutput=False)
    bv_e = nc.declare_dram_parameter("bv", [1, E], F32, isOutput=False)
    wt_e = nc.declare_dram_parameter("wt_out", [S, S], BF16, isOutput=True)
    cx_e = nc.declare_dram_parameter("ctxT_out", [E, S], F32, isOutput=True)


    with tile.TileContext(nc) as tc:
        with tc.tile_pool(name="persist", bufs=1) as persist, \
             tc.tile_pool(name="psA", bufs=1, space="PSUM") as psA:
            qT = [persist.tile([P, S], BF16, tag=f"qT{i}", name=f"qT{i}")
                  for i in range(NE)]
            kT = [persist.tile([P, S], BF16, tag=f"kT{i}", name=f"kT{i}")
                  for i in range(NE)]
            v = [persist.tile([P, E], BF16, tag=f"v{i}", name=f"v{i}")
                 for i in range(NS)]
            pT = [persist.tile([P, S], BF16, tag=f"pT{i}", name=f"pT{i}")
                  for i in range(NS)]

            # ---------------- phase 1: QKV projections ----------------
            # DMA f32 inputs into small staging tiles, DVE-cast to bf16
            # operand tiles (PE runs bf16 at full rate; rounding noise is
            # dominated by the bf16 P^T/weights path anyway).
            with tc.tile_pool(name="ph1", bufs=1) as ph1:
                w_sb = {}
                for nm in ("v", "q", "k"):
                    w_sb[nm] = [ph1.tile([P, E], BF16, tag=f"w{nm}{d}",
                                         name=f"w{nm}{d}") for d in range(ND)]
                xt = [ph1.tile([P, S], BF16, tag=f"xt{i}", name=f"xt{i}")
                      for i in range(ND)]

                def load_cast(dst, src_ap, scalar_eng=False):
                    st = ph1.tile([P, 512], F32, tag="stage", bufs=4)
                    nc.sync.dma_start(out=st, in_=src_ap)
                    if scalar_eng:
                        nc.scalar.copy(out=dst, in_=st)
                    else:
                        nc.vector.tensor_copy(dst, st)

                # wv[d] + xt[d] col-chunk 0 first so V matmuls start early
                for d in range(ND):
                    load_cast(w_sb["v"][d], wv_e.ap()[d * P:(d + 1) * P, :],
                              scalar_eng=True)
                    load_cast(xt[d][:, 0:512],
                              xt_e.ap()[d * P:(d + 1) * P, 0:512])
                bqc = ph1.tile([P, NE], F32, tag="bqc")
                nc.sync.dma_start(out=bqc, in_=bqc_e.ap())
                bkc = ph1.tile([P, NE], F32, tag="bkc")
                nc.sync.dma_start(out=bkc, in_=bkc_e.ap())
                # bv broadcast across all 128 partitions
                bvb = ph1.tile([P, E], F32, tag="bvb")
                src = bv_e.ap()
                nc.sync.dma_start(
                    out=bvb,
                    in_=bass.AP(tensor=src.tensor, offset=src.offset,
                                ap=[[0, P], [1, E]]))
                for cc in range(1, NQ):
                    for dc in range(ND):
                        load_cast(xt[dc][:, cc * 512:(cc + 1) * 512],
                                  xt_e.ap()[dc * P:(dc + 1) * P,
                                            cc * 512:(cc + 1) * 512])
                for nm, we in (("q", wq_e), ("k", wk_e)):
                    for d in range(ND):
                        load_cast(w_sb[nm][d], we.ap()[d * P:(d + 1) * P, :],
                                  scalar_eng=True)

                # V first (scores depend only on Q^T/K^T below)
                for si in range(NS):
                    ps = psA.tile([P, 1024], F32, tag="ps_mm", bufs=4)
                    for dc in range(ND):
                        nc.tensor.matmul(
                            ps[:, :E],
                            lhsT=xt[dc][:, si * P:(si + 1) * P],
                            rhs=w_sb["v"][dc],
                            start=(dc == 0), stop=(dc == ND - 1))
                    nc.vector.tensor_add(v[si], ps[:, :E], bvb)

                # Q^T, K^T in [E, S] layout; bias added on evacuation
                for nm, dst, bc in (("q", qT, bqc), ("k", kT, bkc)):
                    for ei in range(NE):
                        for sj in range(NQ):
                            ps = psA.tile([P, 1024], F32, tag="ps_mm", bufs=4)
                            for dc in range(ND):
                                nc.tensor.matmul(
                                    ps[:, :512],
                                    lhsT=w_sb[nm][dc][:, ei * P:(ei + 1) * P],
                                    rhs=xt[dc][:, sj * 512:(sj + 1) * 512],
                                    start=(dc == 0), stop=(dc == ND - 1))
                            nc.vector.tensor_scalar_add(
                                out=dst[ei][:, sj * 512:(sj + 1) * 512],
                                in0=ps[:, :512],
                                scalar1=bc[:, ei:ei + 1])

            # ---------------- phase 2: scoresT -> P^T ----------------
            with tc.tile_pool(name="ph23", bufs=1) as ph23:
                # running sum of pT tiles (DVE, otherwise idle in phase 2);
                # one gpsimd partition_all_reduce at the end gives row sums
                acc = ph23.tile([P, S], F32, tag="acc")

                def ctxT_mms(halves, ei, kt):
                    for h in range(2):
                        for qj in (2 * h, 2 * h + 1):
                            col = (qj - 2 * h) * 512
                            nc.tensor.matmul(
                                halves[h][:, col:col + 512],
                                lhsT=v[kt][:, ei * P:(ei + 1) * P],
                                rhs=pT[kt][:, qj * 512:(qj + 1) * 512],
                                start=(kt == 0), stop=(kt == NS - 1))

                for kt in range(NS):
                    tt = ph23.tile([P, S], BF16, tag="t_tanh", bufs=2)
                    for h in range(2):
                        ps = psA.tile([P, 1024], F32, tag="ps_mm", bufs=4)
                        for qj in (2 * h, 2 * h + 1):
                            col = (qj - 2 * h) * 512
                            for ec in range(NE):
                                nc.tensor.matmul(
                                    ps[:, col:col + 512],
                                    lhsT=kT[ec][:, kt * P:(kt + 1) * P],
                                    rhs=qT[ec][:, qj * 512:(qj + 1) * 512],
                                    start=(ec == 0), stop=(ec == NE - 1))
                        nc.scalar.activation(
                            tt[:, h * 1024:(h + 1) * 1024], ps,
                            func=AF.Tanh, scale=SCALE)
                    nc.scalar.activation(pT[kt], tt, func=AF.Exp)
                    if kt == 0:
                        nc.vector.tensor_copy(acc, pT[0])
                    else:
                        nc.vector.tensor_add(acc, acc, pT[kt])

                # ------------ softmax denominators (off the PE) ------------
                # two half-width pipelined stages: the first half's recip
                # broadcast lands while the second half's all_reduce runs
                rs_all = ph23.tile([P, S], F32, tag="rs_all")
                rec_row = ph23.tile([1, S], F32, tag="rec_row")
                rec_bf = ph23.tile([1, S], BF16, tag="rec_bf")
                rbc = ph23.tile([P, S], BF16, tag="rbc")
                for h in range(2):
                    sl = slice(h * 1024, (h + 1) * 1024)
                    nc.gpsimd.partition_all_reduce(
                        rs_all[:, sl], acc[:, sl], channels=P,
                        reduce_op=bass_isa.ReduceOp.add)
                    nc.vector.reciprocal_approx_fast(
                        rec_row[:, sl], rs_all[0:1, sl])
                    nc.vector.tensor_copy(rec_bf[:, sl], rec_row[:, sl])
                    nc.gpsimd.partition_broadcast(rbc[:, sl], rec_bf[:, sl])

                # ------------ phase 3: context^T + weights out ------------
                # ctx^T[e, q] = sum_k V[k, e] P^T[k, q]; both outputs
                # normalize along the FREE axis against the rbc broadcast.
                def evac_ctxT(halves, ei):
                    cx_sb = ph23.tile([P, S], F32, tag="ctx_stage", bufs=2,
                                      name=f"cx_sb{ei}")
                    for h in range(2):
                        nc.vector.tensor_mul(
                            cx_sb[:, h * 1024:(h + 1) * 1024], halves[h],
                            rbc[:, h * 1024:(h + 1) * 1024])
                    nc.sync.dma_start(
                        out=cx_e.ap()[ei * P:(ei + 1) * P, :], in_=cx_sb)

                WT_SCHED = {0: range(0, 6), 1: range(6, 12), 2: range(12, 15),
                            3: range(15, 16)}
                for ei in range(NE):
                    halves = [psA.tile([P, 1024], F32, tag="ps_mm", bufs=4,
                                       name=f"psctxT{ei}_{h}") for h in range(2)]
                    for kt in range(NS):
                        ctxT_mms(halves, ei, kt)
                    evac_ctxT(halves, ei)
                    for kt in WT_SCHED[ei]:
                        wt_sb = ph23.tile([P, S], BF16, tag="wt_stage", bufs=3)
                        nc.vector.tensor_mul(wt_sb, pT[kt], rbc)
                        nc.sync.dma_start(
                            out=wt_e.ap()[kt * P:(kt + 1) * P, :], in_=wt_sb)

    nc.finalize()
    return nc


_NC_CACHE = None
_LAST_RESULT = None


def _get_nc():
    global _NC_CACHE
    if _NC_CACHE is None:
        _NC_CACHE = _build()
    return _NC_CACHE


def kernel(x, Wq, bq, Wk, bk, Wv, bv):
    x = np.asarray(x, dtype=np.float32)
    Wq = np.asarray(Wq, dtype=np.float32)
    Wk = np.asarray(Wk, dtype=np.float32)
    Wv = np.asarray(Wv, dtype=np.float32)
    bq = np.asarray(bq, dtype=np.float32).reshape(E)
    bk = np.asarray(bk, dtype=np.float32).reshape(E)
    bv = np.asarray(bv, dtype=np.float32).reshape(1, E)

    nc = _get_nc()
    bqc = np.ascontiguousarray(bq.reshape(NE, P).T)  # [128, 4]
    bkc = np.ascontiguousarray(bk.reshape(NE, P).T)
    shared = {"wq": Wq, "wk": Wk, "wv": Wv, "bqc": bqc, "bkc": bkc, "bv": bv}
    in_maps = [
        {"xt": np.ascontiguousarray(x[b].T), **shared} for b in range(N_CORES)
    ]
    res = run_bass_kernel_spmd(nc, in_maps, list(range(N_CORES)))
    global _LAST_RESULT
    _LAST_RESULT = res

    context = np.stack([
        np.ascontiguousarray(res.results[b]["ctxT_out"].T) for b in range(N_CORES)
    ])
    # weightsT is [k, q] bf16; transpose back and upcast (both exact)
    weights = np.stack([
        np.ascontiguousarray(res.results[b]["wt_out"].T).astype(np.float32)
        for b in range(N_CORES)
    ])
    return context, weights


# revision 24
# speedup vs baseline: 1.0587x; 1.0175x over previous
"""Trainium2 Bass kernel: single-head attention layer (QKV proj + tanh-squashed
scaled-dot-product softmax attention), data-parallel over batch on 8 NeuronCores.

Reference math (per batch b):
    Q = x Wq + bq ; K = x Wk + bk ; V = x Wv + bv          # [S, E]
    scores  = tanh((Q K^T) / sqrt(E))                      # [S, S]
    weights = softmax(scores, axis=-1)
    context = weights V

Device-side strategy (per core, S=2048, D=E=512):
    - host supplies xT = x[b].T [D, S] plus bias re-layouts (layout prep only);
      inputs are DVE-cast to bf16 once on-chip (PE runs bf16 at full rate).
    - V in natural [S, E] layout, Q^T / K^T in [E, S] layout; biases are added
      on the PSUM->SBUF evacuation (DVE), not on the PE.
    - scoresT tiles [128k, S] = K^T.T @ Q^T -> tanh -> exp (ScalarE; softmax
      needs no max subtraction because tanh bounds scores to [-1, 1]).
    - P^T (unnormalized numerator) kept in SBUF as bf16; a running DVE sum of
      P^T tiles + one gpsimd partition_all_reduce gives the softmax
      denominators with zero PE cost; reciprocal_approx_fast + gpsimd
      partition_broadcast make the recip row available on all partitions.
    - both outputs are produced transposed so normalization is a free-axis
      tensor_tensor multiply: weightsT = P^T * rbc (bf16 -> DMA out; host
      transposes + upcasts, both exact), ctx^T = (V.T-chunk @ P^T) * rbc
      (f32 -> DMA out; host transposes).
    - one PSUM pool (4 x [128,1024] rotating buffers, 8 banks) serves QKV,
      scores and ctx^T accumulations — no pool-transition drains on the PE
      critical path; input DMAs are chunked/ordered so the first V matmul
      group unblocks after ~0.5MB.
"""

import sys

sys.path.insert(0, "/opt/trn_rl_repo")

import numpy as np

import concourse.bacc as bacc
import concourse.bass as bass
import concourse.tile as tile
from concourse import bass_isa, mybir
from concourse.bass_utils import run_bass_kernel_spmd

F32 = mybir.dt.float32
BF16 = mybir.dt.bfloat16
AF = mybir.ActivationFunctionType

B = 8          # batch (one per core)
S = 2048       # sequence length
D = 512        # d_in
E = 512        # attention size
P = 128        # SBUF partitions
NS = S // P    # 16 seq tiles of 128
ND = D // P    # 4 contraction chunks
NE = E // P    # 4 e tiles
NQ = S // 512  # 4 moving chunks of 512 along seq
SCALE = 1.0 / float(np.sqrt(E))

N_CORES = 8


def _build():
    nc = bacc.Bacc("TRN2", target_bir_lowering=False, debug=False,
                   num_devices=N_CORES)

    xt_e = nc.declare_dram_parameter("xt", [D, S], F32, isOutput=False)
    wq_e = nc.declare_dram_parameter("wq", [D, E], F32, isOutput=False)
    wk_e = nc.declare_dram_parameter("wk", [D, E], F32, isOutput=False)
    wv_e = nc.declare_dram_parameter("wv", [D, E], F32, isOutput=False)
    # bias re-layouts (host-prepped): [128, 4] with column i = b[128*i:128*(i+1)]
    bqc_e = nc.declare_dram_parameter("bqc", [P, NE], F32, isOutput=False)
    bkc_e = nc.declare_dram_parameter("bkc", [P, NE], F32, isOutput=False)
    bv_e = nc.declare_dram_parameter("bv", [1, E], F32, isOutput=False)
    wt_e = nc.declare_dram_parameter("wt_out", [S, S], BF16, isOutput=True)
    cx_e = nc.declare_dram_parameter("ctxT_out", [E, S], F32, isOutput=True)


    with tile.TileContext(nc) as tc:
        with tc.tile_pool(name="persist", bufs=1) as persist, \
             tc.tile_pool(name="psA", bufs=1, space="PSUM") as psA:
            qT = [persist.tile([P, S], BF16, tag=f"qT{i}", name=f"qT{i}")
                  for i in range(NE)]
            kT = [persist.tile([P, S], BF16, tag=f"kT{i}", name=f"kT{i}")
                  for i in range(NE)]
            v = [persist.tile([P, E], BF16, tag=f"v{i}", name=f"v{i}")
                 for i in range(NS)]
            pT = [persist.tile([P, S], BF16, tag=f"pT{i}", name=f"pT{i}")
                  for i in range(NS)]

            # ---------------- phase 1: QKV projections ----------------
            # DMA f32 inputs into small staging tiles, DVE-cast to bf16
            # operand tiles (PE runs bf16 at full rate; rounding noise is
            # dominated by the bf16 P^T/weights path anyway).
            with tc.tile_pool(name="ph1", bufs=1) as ph1:
                w_sb = {}
                for nm in ("v", "q", "k"):
                    w_sb[nm] = [ph1.tile([P, E], BF16, tag=f"w{nm}{d}",
                                         name=f"w{nm}{d}") for d in range(ND)]
                xt = [ph1.tile([P, S], BF16, tag=f"xt{i}", name=f"xt{i}")
                      for i in range(ND)]

                def load_cast(dst, src_ap, scalar_eng=False):
                    st = ph1.tile([P, 512], F32, tag="stage", bufs=4)
                    nc.sync.dma_start(out=st, in_=src_ap)
                    if scalar_eng:
                        nc.scalar.copy(out=dst, in_=st)
                    else:
                        nc.vector.tensor_copy(dst, st)

                # wv[d] + xt[d] col-chunk 0 first so V matmuls start early
                for d in range(ND):
                    load_cast(w_sb["v"][d], wv_e.ap()[d * P:(d + 1) * P, :],
                              scalar_eng=True)
                    load_cast(xt[d][:, 0:512],
                              xt_e.ap()[d * P:(d + 1) * P, 0:512])
                bqc = ph1.tile([P, NE], F32, tag="bqc")
                nc.sync.dma_start(out=bqc, in_=bqc_e.ap())
                bkc = ph1.tile([P, NE], F32, tag="bkc")
                nc.sync.dma_start(out=bkc, in_=bkc_e.ap())
                # bv broadcast across all 128 partitions
                bvb = ph1.tile([P, E], F32, tag="bvb")
                src = bv_e.ap()
                nc.sync.dma_start(
                    out=bvb,
                    in_=bass.AP(tensor=src.tensor, offset=src.offset,
                                ap=[[0, P], [1, E]]))
                for cc in range(1, NQ):
                    for dc in range(ND):
                        load_cast(xt[dc][:, cc * 512:(cc + 1) * 512],
                                  xt_e.ap()[dc * P:(dc + 1) * P,
                                            cc * 512:(cc + 1) * 512])
                for nm, we in (("q", wq_e), ("k", wk_e)):
                    for d in range(ND):
                        load_cast(w_sb[nm][d], we.ap()[d * P:(d + 1) * P, :],
                                  scalar_eng=True)

                # V first (scores depend only on Q^T/K^T below)
                for si in range(NS):
                    ps = psA.tile([P, 1024], F32, tag="ps_mm", bufs=4)
                    for dc in range(ND):
                        nc.tensor.matmul(
                            ps[:, :E],
                            lhsT=xt[dc][:, si * P:(si + 1) * P],
                            rhs=w_sb["v"][dc],
                            start=(dc == 0), stop=(dc == ND - 1))
                    nc.vector.tensor_add(v[si], ps[:, :E], bvb)

                # Q^T, K^T in [E, S] layout; bias added on evacuation
                for nm, dst, bc in (("q", qT, bqc), ("k", kT, bkc)):
                    for ei in range(NE):
                        for sj in range(NQ):
                            ps = psA.tile([P, 1024], F32, tag="ps_mm", bufs=4)
                            for dc in range(ND):
                                nc.tensor.matmul(
                                    ps[:, :512],
                                    lhsT=w_sb[nm][dc][:, ei * P:(ei + 1) * P],
                                    rhs=xt[dc][:, sj * 512:(sj + 1) * 512],
                                    start=(dc == 0), stop=(dc == ND - 1))
                            nc.vector.tensor_scalar_add(
                                out=dst[ei][:, sj * 512:(sj + 1) * 512],
                                in0=ps[:, :512],
                                scalar1=bc[:, ei:ei + 1])

            # ---------------- phase 2: scoresT -> P^T ----------------
            with tc.tile_pool(name="ph23", bufs=1) as ph23:
                # running sum of pT tiles (DVE, otherwise idle in phase 2);
                # one gpsimd partition_all_reduce at the end gives row sums
                acc = ph23.tile([P, S], F32, tag="acc")

                def ctxT_mms(halves, ei, kt):
                    for h in range(2):
                        for qj in (2 * h, 2 * h + 1):
                            col = (qj - 2 * h) * 512
                            nc.tensor.matmul(
                                halves[h][:, col:col + 512],
                                lhsT=v[kt][:, ei * P:(ei + 1) * P],
                                rhs=pT[kt][:, qj * 512:(qj + 1) * 512],
                                start=(kt == 0), stop=(kt == NS - 1))

                for kt in range(NS):
                    tt = ph23.tile([P, S], BF16, tag="t_tanh", bufs=2)
                    for h in range(2):
                        ps = psA.tile([P, 1024], F32, tag="ps_mm", bufs=4)
                        for qj in (2 * h, 2 * h + 1):
                            col = (qj - 2 * h) * 512
                            for ec in range(NE):
                                nc.tensor.matmul(
                                    ps[:, col:col + 512],
                                    lhsT=kT[ec][:, kt * P:(kt + 1) * P],
                                    rhs=qT[ec][:, qj * 512:(qj + 1) * 512],
                                    start=(ec == 0), stop=(ec == NE - 1))
                        nc.scalar.activation(
                            tt[:, h * 1024:(h + 1) * 1024], ps,
                            func=AF.Tanh, scale=SCALE)
                    nc.scalar.activation(pT[kt], tt, func=AF.Exp)
                    if kt == 0:
                        nc.vector.tensor_copy(acc, pT[0])
                    else:
                        nc.vector.tensor_add(acc, acc, pT[kt])
                    if kt == NS - 2:
                        # warm the gpsimd Q7 reduce/broadcast handlers so the
                        # real chain below skips the ~9.5us cold dispatch
                        wsrc = ph23.tile([P, 4], F32, tag="warm_src")
                        nc.vector.memset(wsrc, 1.0)
                        wdst = ph23.tile([P, 4], F32, tag="warm_dst")
                        nc.gpsimd.partition_all_reduce(
                            wdst, wsrc, channels=P,
                            reduce_op=bass_isa.ReduceOp.add)
                        nc.gpsimd.partition_broadcast(wdst, wsrc[0:1, :])

                # ------------ softmax denominators (off the PE) ------------
                # two half-width pipelined stages: the first half's recip
                # broadcast lands while the second half's all_reduce runs
                rs_all = ph23.tile([P, S], F32, tag="rs_all")
                rec_row = ph23.tile([1, S], F32, tag="rec_row")
                rec_bf = ph23.tile([1, S], BF16, tag="rec_bf")
                rbc = ph23.tile([P, S], BF16, tag="rbc")
                for h in range(2):
                    sl = slice(h * 1024, (h + 1) * 1024)
                    nc.gpsimd.partition_all_reduce(
                        rs_all[:, sl], acc[:, sl], channels=P,
                        reduce_op=bass_isa.ReduceOp.add)
                    nc.vector.reciprocal_approx_fast(
                        rec_row[:, sl], rs_all[0:1, sl])
                    nc.vector.tensor_copy(rec_bf[:, sl], rec_row[:, sl])
                    nc.gpsimd.partition_broadcast(rbc[:, sl], rec_bf[:, sl])

                # ------------ phase 3: context^T + weights out ------------
                # ctx^T[e, q] = sum_k V[k, e] P^T[k, q]; both outputs
                # normalize along the FREE axis against the rbc broadcast.
                def evac_ctxT(halves, ei):
                    cx_sb = ph23.tile([P, S], F32, tag="ctx_stage", bufs=2,
                                      name=f"cx_sb{ei}")
                    for h in range(2):
                        nc.vector.tensor_mul(
                            cx_sb[:, h * 1024:(h + 1) * 1024], halves[h],
                            rbc[:, h * 1024:(h + 1) * 1024])
                    nc.sync.dma_start(
                        out=cx_e.ap()[ei * P:(ei + 1) * P, :], in_=cx_sb)

                WT_SCHED = {0: range(0, 6), 1: range(6, 12), 2: range(12, 15),
                            3: range(15, 16)}
                for ei in range(NE):
                    halves = [psA.tile([P, 1024], F32, tag="ps_mm", bufs=4,
                                       name=f"psctxT{ei}_{h}") for h in range(2)]
                    for kt in range(NS):
                        ctxT_mms(halves, ei, kt)
                    evac_ctxT(halves, ei)
                    for kt in WT_SCHED[ei]:
                        wt_sb = ph23.tile([P, S], BF16, tag="wt_stage", bufs=3)
                        nc.vector.tensor_mul(wt_sb, pT[kt], rbc)
                        nc.sync.dma_start(
                            out=wt_e.ap()[kt * P:(kt + 1) * P, :], in_=wt_sb)

    nc.finalize()
    return nc


_NC_CACHE = None
_LAST_RESULT = None


def _get_nc():
    global _NC_CACHE
    if _NC_CACHE is None:
        _NC_CACHE = _build()
    return _NC_CACHE


def kernel(x, Wq, bq, Wk, bk, Wv, bv):
    x = np.asarray(x, dtype=np.float32)
    Wq = np.asarray(Wq, dtype=np.float32)
    Wk = np.asarray(Wk, dtype=np.float32)
    Wv = np.asarray(Wv, dtype=np.float32)
    bq = np.asarray(bq, dtype=np.float32).reshape(E)
    bk = np.asarray(bk, dtype=np.float32).reshape(E)
    bv = np.asarray(bv, dtype=np.float32).reshape(1, E)

    nc = _get_nc()
    bqc = np.ascontiguousarray(bq.reshape(NE, P).T)  # [128, 4]
    bkc = np.ascontiguousarray(bk.reshape(NE, P).T)
    shared = {"wq": Wq, "wk": Wk, "wv": Wv, "bqc": bqc, "bkc": bkc, "bv": bv}
    in_maps = [
        {"xt": np.ascontiguousarray(x[b].T), **shared} for b in range(N_CORES)
    ]
    res = run_bass_kernel_spmd(nc, in_maps, list(range(N_CORES)))
    global _LAST_RESULT
    _LAST_RESULT = res

    context = np.stack([
        np.ascontiguousarray(res.results[b]["ctxT_out"].T) for b in range(N_CORES)
    ])
    # weightsT is [k, q] bf16; transpose back and upcast (both exact)
    weights = np.stack([
        np.ascontiguousarray(res.results[b]["wt_out"].T).astype(np.float32)
        for b in range(N_CORES)
    ])
    return context, weights
